# revision 9
# baseline (speedup 1.0000x reference)
"""Trainium2 Bass kernel for 4D convolution (3x3x3x3, pad 1, stride 1).

Problem: x (2, 8, 7, 7, 48, 48) f32, conv (8, 648) f32, bias (8,) f32
         -> out (2, 8, 7, 7, 48, 48) f32.

Sharding: 8 cores = (batch b in {0,1}) x (h-chunk hc in {0..3}, 12 rows).

Per core: 16-way 32x32 tile_position packing of the PE array.

The (s -> t) banded weight structure (output h-row t contracts input
h-rows s in {t-1, t, t+1}, 8 channels each) is covered exactly by four
32x32 rectangles, each 9 dense 8x8 blocks:

  R0: T={0,1,2,3}   S={-1,0,1,2}   cols [t2,t3,t0,t1]
  R1: T={2,3,4,5}   S={3,4,5,6}    cols [t2,t3,t4,t5]
  R2: T={6,7,8,9}   S={5,6,7,8}    cols [t8,t9,t6,t7]
  R3: T={8,9,10,11} S={9,10,11,12} cols [t8,t9,t10,t11]

One task (u, shift) = 4 matmuls at tile_position (32r, 32k), k = lane.
All four rectangles of a task stream the SAME rhs free-columns (only
the partition base differs), so the column-group's XBUS serves all
four row-tiles with one synchronized stream; each rectangle writes a
DIFFERENT psum bank (col offset 512*(4*slot+r)) of the lane's
partition group, avoiding the row-tile same-bank conflict.  Four lanes
(= column groups) run one task each concurrently: 16 K=32 M=32 N=336
tiles in flight.

Lane u-assignments (slot 0, slot 1): (u1, u0) (u2, u5a) (u3, u6)
(u4, u5b), where u5's 27 shifts split 18/9 across lanes 1/3 for load
balance (45/45/45/36 tasks); the host adds the two u5 pieces.

Slot-0 finishes on all four lanes at the same step, so one 128-row
ACT evacuates u1..u4 psum together; slot-1 drains per-lane on
ACT/DVE.  Host adds the split t2/t3 + t8/t9 rectangle pairs, bias,
and gathers the t-order.  SBUF tiles are double-buffered so the
hardware reps loop pipelines DMA-in and drain tails across
iterations.
"""

import sys

if "/opt/trn_rl_repo" not in sys.path:
    sys.path.insert(0, "/opt/trn_rl_repo")

import numpy as np
import ml_dtypes

B, C, OC = 2, 8, 8
U, V, H, W = 7, 7, 48, 48
TH = 12
NCHUNKS = H // TH
NCORES = B * NCHUNKS
NCOL = V * W        # 336
XROW = (V + 2) * (W + 2)  # 450
XFREE = U * XROW

# rectangle r: (T list in column order, S list in partition order)
RECTS = [
    ([2, 3, 0, 1], [-1, 0, 1, 2]),
    ([2, 3, 4, 5], [3, 4, 5, 6]),
    ([8, 9, 6, 7], [5, 6, 7, 8]),
    ([8, 9, 10, 11], [9, 10, 11, 12]),
]

SH_ORDER = [
    (i0, i1, i3) for i0 in (1, 2, 0) for i1 in range(3) for i3 in range(3)
]

# lane k slot s holds (u, shift-subset): u5 split 18/9 over lanes 1, 3
LANE_SLOTS = [
    [(1, 0, 27), (0, 0, 27)],
    [(2, 0, 27), (5, 0, 18)],
    [(3, 0, 27), (6, 0, 27)],
    [(4, 0, 27), (5, 18, 27)],
]

_built = {}


def _build_nc(reps=None):
    import contextlib

    import concourse.bacc as bacc
    import concourse.mybir as mybir
    from concourse.tile import TileContext

    BF16 = mybir.dt.bfloat16
    F32 = mybir.dt.float32

    nc = bacc.Bacc(
        "TRN2", target_bir_lowering=False, debug=False, num_devices=NCORES
    )
    xw_d = nc.dram_tensor("xw", [128, XFREE], BF16, kind="ExternalInput")
    wt_d = nc.dram_tensor("wt", [128, 27 * 32], BF16, kind="ExternalInput")
    # out: per (lane, slot): 32 partitions x (4 rects, 336) bf16
    out_d = nc.dram_tensor("out", [128, 2 * 4 * NCOL], BF16,
                           kind="ExternalOutput")

    with TileContext(nc) as tc:
        with (
            tc.tile_pool(name="sbuf", bufs=1) as pool,
            tc.tile_pool(name="psum", bufs=1, space="PSUM") as pp,
        ):
            loop = tc.For_i(0, reps, 1) if reps is not None else contextlib.nullcontext()
            with loop:
                scr = pool.tile([128, 64], BF16, tag="scr", bufs=2)
                nc.gpsimd.memset(scr[:], 0.0)
                # one psum tile = all 8 banks; lane k partitions 32k..+31,
                # bank (4*slot + r) cols 512*(4*slot+r)..+335
                ps = pp.tile([128, 4096], F32, tag="ps", bufs=1, name="ps")
                for wi in range(2):
                    nc.tensor.matmul(
                        ps[96:128, 2048:2112], scr[0:32, :32],
                        scr[0:32, :64], start=True, stop=True,
                        tile_position=(0, 96),
                    )

                w_first = pool.tile([128, 32], BF16, tag="wf", bufs=2,
                                    name="w_first")
                w_sb = pool.tile([128, 26 * 32], BF16, tag="w", bufs=2,
                                 name="w_sb")
                x_sb = pool.tile([128, XFREE], BF16, tag="x", bufs=2,
                                 name="x_sb")
                o_sb = pool.tile([128, 2 * 4 * NCOL], BF16, tag="o", bufs=2,
                                 name="o_sb")
                # x windows: step-0 tasks need XROWs 1-4; 5 and 0 by step
                # 9/18; 6 from step 27 (slot 1)
                nc.sync.dma_start(
                    out=x_sb[:, 1 * XROW : 5 * XROW],
                    in_=xw_d[:, 1 * XROW : 5 * XROW],
                )
                nc.gpsimd.dma_start(out=w_first[:], in_=wt_d[:, 0:32])
                nc.gpsimd.dma_start(
                    out=x_sb[:, 5 * XROW : 6 * XROW],
                    in_=xw_d[:, 5 * XROW : 6 * XROW],
                )
                nc.sync.dma_start(
                    out=x_sb[:, 0 : 1 * XROW], in_=xw_d[:, 0 : 1 * XROW]
                )
                nc.gpsimd.dma_start(out=w_sb[:], in_=wt_d[:, 32:])
                nc.sync.dma_start(
                    out=x_sb[:, 6 * XROW :], in_=xw_d[:, 6 * XROW :]
                )

                def lhsT_for(pos, r):
                    rows = slice(32 * r, 32 * r + 32)
                    if pos == 0:
                        return w_first[rows, :]
                    return w_sb[rows, (pos - 1) * 32 : pos * 32]

                def rhs_for(u, i0, i1, i3, r):
                    return (
                        x_sb[
                            32 * r : 32 * r + 32,
                            (u + i0 - 1) * XROW : (u + i0) * XROW,
                        ]
                        .rearrange("p (v w) -> p v w", v=V + 2)
                        [:, i1 : i1 + V, i3 : i3 + W]
                    )

                # per-lane task lists: (slot, pos, i0, i1, i3, first, last)
                lane_tasks = []
                for k in range(4):
                    tasks = []
                    for slot, (u, lo, hi) in enumerate(LANE_SLOTS[k]):
                        shifts = [
                            (pos, i0, i1, i3)
                            for pos, (i0, i1, i3) in enumerate(SH_ORDER)
                            if 1 <= u + i0 <= 7
                        ][lo:hi]
                        for idx, (pos, i0, i1, i3) in enumerate(shifts):
                            tasks.append((
                                u, slot, pos, i0, i1, i3,
                                idx == 0, idx == len(shifts) - 1,
                            ))
                    lane_tasks.append(tasks)

                def psum_3d(p0, np_, slot):
                    return (
                        ps[p0 : p0 + np_, 2048 * slot : 2048 * slot + 2048]
                        .rearrange("p (r n) -> p r n", r=4)[:, :, 0:NCOL]
                    )

                def osb_3d(p0, np_, slot):
                    return (
                        o_sb[
                            p0 : p0 + np_,
                            4 * NCOL * slot : 4 * NCOL * slot + 4 * NCOL,
                        ].rearrange("p (r n) -> p r n", r=4)
                    )

                def drain_quad_slot0():
                    nc.scalar.activation(
                        out=osb_3d(0, 128, 0),
                        in_=psum_3d(0, 128, 0),
                        func=mybir.ActivationFunctionType.Identity,
                    )
                    nc.sync.dma_start(
                        out=out_d[:, 0 : 4 * NCOL],
                        in_=o_sb[:, 0 : 4 * NCOL],
                    )

                def drain_slot1(k):
                    if k in (0, 2):
                        nc.vector.tensor_scalar_add(
                            out=osb_3d(32 * k, 32, 1),
                            in0=psum_3d(32 * k, 32, 1),
                            scalar1=0.0,
                        )
                    else:
                        nc.scalar.activation(
                            out=osb_3d(32 * k, 32, 1),
                            in_=psum_3d(32 * k, 32, 1),
                            func=mybir.ActivationFunctionType.Identity,
                        )
                    ring = nc.gpsimd if k in (0, 1) else nc.sync
                    ring.dma_start(
                        out=out_d[32 * k : 32 * k + 32, 4 * NCOL :],
                        in_=o_sb[32 * k : 32 * k + 32, 4 * NCOL :],
                    )

                nsteps = max(len(t) for t in lane_tasks)
                for step in range(nsteps):
                    for k in range(4):
                        if step >= len(lane_tasks[k]):
                            continue
                        u, slot, pos, i0, i1, i3, first, last = \
                            lane_tasks[k][step]
                        for r in range(4):
                            nc.tensor.matmul(
                                ps[
                                    32 * k : 32 * k + 32,
                                    512 * (4 * slot + r)
                                    : 512 * (4 * slot + r) + NCOL,
                                ],
                                lhsT_for(pos, r),
                                rhs_for(u, i0, i1, i3, r),
                                start=first,
                                stop=last,
                                tile_position=(32 * r, 32 * k),
                            )
                        if last and slot == 0 and k == 3:
                            drain_quad_slot0()
                        if last and slot == 1:
                            drain_slot1(k)

    nc.compile()
    return nc


def _get_nc():
    if "nc" not in _built:
        _built["nc"] = _build_nc()
    return _built["nc"]


def _build_weight_inputs(conv):
    Wr = conv.reshape(OC, 3, 3, 3, 3, C).astype(np.float32)
    # wt[32r + 8*si + c, pos*32 + 8*tj + o] = Wr[o,i0,i1,i2,i3,c],
    # i2 = S_r[si] - T_r[tj] + 1 when in 0..2, else 0
    wt = np.zeros((128, 27, 32), np.float32)
    for r, (tlist, slist) in enumerate(RECTS):
        for si, s in enumerate(slist):
            for tj, t in enumerate(tlist):
                i2 = s - t + 1
                if 0 <= i2 <= 2:
                    for pos, (i0, i1, i3) in enumerate(SH_ORDER):
                        p0 = 32 * r + 8 * si
                        wt[p0 : p0 + 8, pos, 8 * tj : 8 * tj + 8] = Wr[
                            :, i0, i1, i2, i3, :
                        ].T
    return np.ascontiguousarray(
        wt.reshape(128, 27 * 32).astype(ml_dtypes.bfloat16)
    )


def _build_x_inputs(x):
    xh = np.zeros((B, C, U, V, H + 2, W), np.float32)
    xh[:, :, :, :, 1 : H + 1, :] = x
    xs = []
    for core in range(NCORES):
        b, hc = divmod(core, NCHUNKS)
        # slab rows s=-1..12 map to padded indices hc*TH + (s+1)
        slab = xh[b, :, :, :, hc * TH : hc * TH + TH + 2, :]  # (C,U,V,14,W)
        xc = np.zeros((C, TH + 2, U, V + 2, W + 2), np.float32)
        xc[:, :, :, 1 : V + 1, 1 : W + 1] = slab.transpose(0, 3, 1, 2, 4)
        sm = xc.transpose(1, 0, 2, 3, 4)  # (14, C, U, V+2, W+2)
        x128 = np.empty((128, XFREE), np.float32)
        for r, (_, slist) in enumerate(RECTS):
            for si, s in enumerate(slist):
                p0 = 32 * r + 8 * si
                x128[p0 : p0 + 8] = sm[s + 1].reshape(C, XFREE)
        xs.append(
            np.ascontiguousarray(x128.astype(ml_dtypes.bfloat16))
        )
    return xs


# u -> list of (lane, slot) pieces
PIECES_OF_U = {}
for _k, _slots in enumerate(LANE_SLOTS):
    for _s, (_u, _, _) in enumerate(_slots):
        PIECES_OF_U.setdefault(_u, []).append((_k, _s))


def kernel(x, conv, bias):
    from concourse.bass_utils import run_bass_kernel_spmd

    nc = _get_nc()
    wt = _build_weight_inputs(np.asarray(conv))
    xs = _build_x_inputs(np.asarray(x, dtype=np.float32))
    in_maps = [{"xw": xc, "wt": wt} for xc in xs]
    res = run_bass_kernel_spmd(nc, in_maps, core_ids=list(range(NCORES)))

    bias = np.asarray(bias, dtype=np.float32)
    out = np.empty((B, OC, U, V, H, W), np.float32)
    for core in range(NCORES):
        b, hc = divmod(core, NCHUNKS)
        raw = np.asarray(res.results[core]["out"], dtype=np.float32)
        raw = raw.reshape(128, 2, 4, V, W)  # partitions, slot, rect, v, w
        for u in range(U):
            blk = np.zeros((32, 4, V, W), np.float32)
            for (k, s) in PIECES_OF_U[u]:
                blk = blk + raw[32 * k : 32 * k + 32, s]
            # rect r rows: 4 t-blocks of 8 rows each, col order per RECTS;
            # merge split pairs: t2,t3 = R0[0:16]+R1[0:16];
            #                    t8,t9 = R2[0:16]+R3[0:16]
            r8 = blk.reshape(4, 8, 4, V, W)  # tblk, o, rect, v, w
            rows = np.empty((TH, OC, V, W), np.float32)
            rows[2] = r8[0, :, 0] + r8[0, :, 1]
            rows[3] = r8[1, :, 0] + r8[1, :, 1]
            rows[0] = r8[2, :, 0]
            rows[1] = r8[3, :, 0]
            rows[4] = r8[2, :, 1]
            rows[5] = r8[3, :, 1]
            rows[8] = r8[0, :, 2] + r8[0, :, 3]
            rows[9] = r8[1, :, 2] + r8[1, :, 3]
            rows[6] = r8[2, :, 2]
            rows[7] = r8[3, :, 2]
            rows[10] = r8[2, :, 3]
            rows[11] = r8[3, :, 3]
            out[b, :, u, :, hc * TH : hc * TH + TH, :] = (
                rows.transpose(1, 2, 0, 3) + bias[:, None, None, None]
            )
    return out


# revision 10
# speedup vs baseline: 1.1214x; 1.1214x over previous
"""Trainium2 Bass kernel for 4D convolution (3x3x3x3, pad 1, stride 1).

Problem: x (2, 8, 7, 7, 48, 48) f32, conv (8, 648) f32, bias (8,) f32
         -> out (2, 8, 7, 7, 48, 48) f32.

Sharding: 8 cores = (batch b in {0,1}) x (h-chunk hc in {0..3}, 12 rows).

Per core: one fat banded matmul per (u, shift): K=112 (14 h-window
rows x 8 channels, s-major partitions 0-111), M=96 (12 output h-rows
x 8 output channels), N=336 (the (v,w) plane).  The measured cost law
on this backend is time ~ sum of output free-dim sizes over matmul
instructions (K, M, and tile packing are free), so the minimal-cost
mapping is the fewest, widest matmuls: 171 valid (u, shift) pairs x
336 columns.

Each u accumulates its 27 (or 18 at the u edges) shift contributions
into its own PSUM bank (7 banks), drained by a single 96-row ACT pass
(psum -> bf16, bias added on host) and DMA'd out per u, u-major, so
drains and DMAs stagger and overlap compute.  Weight and x DMAs are
chunked across two rings so the hardware reps loop pipelines
cross-iteration (each chunk's WAR frees mid-iteration).
"""

import sys

if "/opt/trn_rl_repo" not in sys.path:
    sys.path.insert(0, "/opt/trn_rl_repo")

import numpy as np
import ml_dtypes

B, C, OC = 2, 8, 8
U, V, H, W = 7, 7, 48, 48
TH = 12
S = TH + 2          # 14 h-window rows
K = S * C           # 112 contraction
M = TH * OC         # 96 outputs
NCHUNKS = H // TH
NCORES = B * NCHUNKS
NCOL = V * W        # 336
XROW = (V + 2) * (W + 2)  # 450
XFREE = U * XROW

SH_ORDER = [
    (i0, i1, i3) for i0 in (1, 2, 0) for i1 in range(3) for i3 in range(3)
]

_built = {}


def _build_nc(reps=None):
    import contextlib

    import concourse.bacc as bacc
    import concourse.mybir as mybir
    from concourse.tile import TileContext

    BF16 = mybir.dt.bfloat16
    F32 = mybir.dt.float32

    nc = bacc.Bacc(
        "TRN2", target_bir_lowering=False, debug=False, num_devices=NCORES
    )
    xw_d = nc.dram_tensor("xw", [128, XFREE], BF16, kind="ExternalInput")
    wt_d = nc.dram_tensor("wt", [128, 27 * M], BF16, kind="ExternalInput")
    out_d = nc.dram_tensor("out", [M, U * NCOL], BF16, kind="ExternalOutput")

    # weight chunks (pos ranges) and x chunks (XROW ranges), sized so each
    # chunk's last reader sits mid-iteration and the re-DMA hides
    W_CHUNKS = [(0, 7), (7, 14), (14, 21), (21, 27)]
    X_CHUNKS = [(0, 2), (2, 4), (4, 6), (6, 7)]

    with TileContext(nc) as tc:
        with (
            tc.tile_pool(name="sbuf", bufs=1) as pool,
            tc.tile_pool(name="psum", bufs=1, space="PSUM") as pp,
        ):
            loop = tc.For_i(0, reps, 1) if reps is not None else contextlib.nullcontext()
            with loop:
                scr = pool.tile([128, 64], BF16, tag="scr")
                nc.gpsimd.memset(scr[:], 0.0)
                # 8 psum banks: bank u (cols 512u..) accumulates u's plane;
                # bank 7 takes the warmups
                ps = pp.tile([128, 4096], F32, tag="ps", bufs=1, name="ps")
                for wi in range(2):
                    nc.tensor.matmul(
                        ps[0:32, 3584:3648], scr[0:32, :32],
                        scr[0:32, :64], start=True, stop=True,
                        tile_position=(0, 0),
                    )

                w_sb = pool.tile([128, 27 * M], BF16, tag="w", name="w_sb")
                x_sb = pool.tile([128, XFREE], BF16, tag="x", name="x_sb")
                o_sb = pool.tile([128, U * NCOL], BF16, tag="o", name="o_sb")
                for ci, (lo, hi) in enumerate(W_CHUNKS):
                    nc.gpsimd.dma_start(
                        out=w_sb[:, lo * M : hi * M],
                        in_=wt_d[:, lo * M : hi * M],
                    )
                for ci, (lo, hi) in enumerate(X_CHUNKS):
                    nc.sync.dma_start(
                        out=x_sb[:, lo * XROW : hi * XROW],
                        in_=xw_d[:, lo * XROW : hi * XROW],
                    )

                def rhs_for(u, i0, i1, i3):
                    return (
                        x_sb[
                            0:K,
                            (u + i0 - 1) * XROW : (u + i0) * XROW,
                        ]
                        .rearrange("p (v w) -> p v w", v=V + 2)
                        [:, i1 : i1 + V, i3 : i3 + W]
                    )

                for u in range(U):
                    shifts = [
                        (pos, i0, i1, i3)
                        for pos, (i0, i1, i3) in enumerate(SH_ORDER)
                        if 1 <= u + i0 <= 7
                    ]
                    n = len(shifts)
                    for idx, (pos, i0, i1, i3) in enumerate(shifts):
                        nc.tensor.matmul(
                            ps[0:M, 512 * u : 512 * u + NCOL],
                            w_sb[0:K, pos * M : (pos + 1) * M],
                            rhs_for(u, i0, i1, i3),
                            start=(idx == 0),
                            stop=(idx == n - 1),
                        )
                    ucols = slice(u * NCOL, (u + 1) * NCOL)
                    nc.scalar.activation(
                        out=o_sb[0:M, ucols],
                        in_=ps[0:M, 512 * u : 512 * u + NCOL],
                        func=mybir.ActivationFunctionType.Identity,
                    )
                    ring = nc.sync if u % 2 == 0 else nc.gpsimd
                    ring.dma_start(
                        out=out_d[:, ucols], in_=o_sb[0:M, ucols]
                    )

    nc.compile()
    return nc


def _get_nc():
    if "nc" not in _built:
        _built["nc"] = _build_nc()
    return _built["nc"]


def _build_weight_inputs(conv):
    Wr = conv.reshape(OC, 3, 3, 3, 3, C).astype(np.float32)
    # wt[s_idx*8 + c, pos*96 + t*8 + o] = Wr[o,i0,i1,i2,i3,c]
    # with i2 = (s_idx - 1) - t + 1, valid 0..2
    wt = np.zeros((128, 27, M), np.float32)
    for s_idx in range(S):
        for t in range(TH):
            i2 = s_idx - t
            if 0 <= i2 <= 2:
                for pos, (i0, i1, i3) in enumerate(SH_ORDER):
                    p0 = s_idx * 8
                    wt[p0 : p0 + 8, pos, t * 8 : t * 8 + 8] = Wr[
                        :, i0, i1, i2, i3, :
                    ].T
    return np.ascontiguousarray(
        wt.reshape(128, 27 * M).astype(ml_dtypes.bfloat16)
    )


def _build_x_inputs(x):
    xh = np.zeros((B, C, U, V, H + 2, W), np.float32)
    xh[:, :, :, :, 1 : H + 1, :] = x
    xs = []
    for core in range(NCORES):
        b, hc = divmod(core, NCHUNKS)
        slab = xh[b, :, :, :, hc * TH : hc * TH + S, :]  # (C,U,V,14,W)
        xc = np.zeros((C, S, U, V + 2, W + 2), np.float32)
        xc[:, :, :, 1 : V + 1, 1 : W + 1] = slab.transpose(0, 3, 1, 2, 4)
        sm = xc.transpose(1, 0, 2, 3, 4)  # (14, C, U, V+2, W+2)
        x128 = np.zeros((128, XFREE), np.float32)
        x128[0:K] = sm.reshape(K, XFREE)
        xs.append(
            np.ascontiguousarray(x128.astype(ml_dtypes.bfloat16))
        )
    return xs


def kernel(x, conv, bias):
    from concourse.bass_utils import run_bass_kernel_spmd

    nc = _get_nc()
    wt = _build_weight_inputs(np.asarray(conv))
    xs = _build_x_inputs(np.asarray(x, dtype=np.float32))
    in_maps = [{"xw": xc, "wt": wt} for xc in xs]
    res = run_bass_kernel_spmd(nc, in_maps, core_ids=list(range(NCORES)))

    bias = np.asarray(bias, dtype=np.float32)
    out = np.empty((B, OC, U, V, H, W), np.float32)
    for core in range(NCORES):
        b, hc = divmod(core, NCHUNKS)
        # out rows: t*8 + o
        r = np.asarray(
            res.results[core]["out"], dtype=np.float32
        ).reshape(TH, OC, U, V, W)
        out[b, :, :, :, hc * TH : (hc + 1) * TH, :] = (
            r.transpose(1, 2, 3, 0, 4) + bias[:, None, None, None, None]
        )
    return out


# revision 14
# speedup vs baseline: 1.1689x; 1.0423x over previous
"""Trainium2 Bass kernel for 4D convolution (3x3x3x3, pad 1, stride 1),
Winograd F(2,3) along the w axis.

Problem: x (2, 8, 7, 7, 48, 48) f32, conv (8, 648) f32, bias (8,) f32
         -> out (2, 8, 7, 7, 48, 48) f32.

Sharding: 8 cores = (batch b in {0,1}) x (h-chunk hc in {0..3}, 12 rows).

The w-axis 3-tap conv is computed as Winograd F(2,3): 24 tiles of 2
outputs each, 4 transformed points per tile.  The host precomputes the
input transform x_hat[q] (B^T-combinations of w-neighbours, pure input
marshaling) and the weight transform (G-combinations over i3).  The
device then runs, per (u, i0, i1, q, half), a banded matmul K=64
(8 h-window rows x 8 ch), M=48 (6 output h-rows x 8 out-ch), N=168
(7 v x 24 w-tiles), halves on disjoint PE row-group pairs streaming
concurrently (the proven 2-half choreography).  Columns streamed per
(u,i0,i1): 4 x 168 = 672 vs 3 x 336 = 1008 spatial -> 2/3 the PE time.

PSUM per u: [128, 2048]: half h banks 2h..2h+1, point q at col offset
h*1024 + (q//2)*512 + (q%2)*168 — uniform (2,2,168) nested strides, so
one ACT (h0) + one DVE (h1) op evacuates all 4 q-planes per half.
The inverse transform out_even = y0+y1+y2, out_odd = y1-y2-y3 and the
bias add run on the host.
"""

import sys

if "/opt/trn_rl_repo" not in sys.path:
    sys.path.insert(0, "/opt/trn_rl_repo")

import numpy as np
import ml_dtypes

B, C, OC = 2, 8, 8
U, V, H, W = 7, 7, 48, 48
TH = 12
THH = 6             # rows per half
S = TH + 2
SH = THH + 2        # window rows per half
KH = C * SH         # 64  contraction per half
MH = OC * THH       # 48  outputs per half
NCHUNKS = H // TH
NCORES = B * NCHUNKS
NT = W // 2         # 24 w-tiles
NQ = 4              # winograd points
NW = V * NT         # 168 columns per point-matmul
XROW = (V + 2) * NQ * NT  # 864 free elems per (partition, u-window)
XFREE = U * XROW

W_ORDER = [(i0, i1) for i0 in (1, 2, 0) for i1 in range(3)]

_built = {}


def _build_nc(reps=None):
    import contextlib

    import concourse.bacc as bacc
    import concourse.mybir as mybir
    from concourse.tile import TileContext

    BF16 = mybir.dt.bfloat16
    F32 = mybir.dt.float32

    nc = bacc.Bacc(
        "TRN2", target_bir_lowering=False, debug=False, num_devices=NCORES
    )
    xw_d = nc.dram_tensor("xw", [128, XFREE], BF16, kind="ExternalInput")
    wt_d = nc.dram_tensor("wt", [128, 36 * MH], BF16, kind="ExternalInput")
    out_d = nc.dram_tensor("out", [128, U * NQ * NW], BF16,
                           kind="ExternalOutput")

    with TileContext(nc) as tc:
        with (
            tc.tile_pool(name="sbuf", bufs=1) as pool,
            tc.tile_pool(name="psum", bufs=1, space="PSUM") as pp,
        ):
            loop = tc.For_i(0, reps, 1) if reps is not None else contextlib.nullcontext()
            with loop:
                # psum: tag ps0 for even u, ps1 for odd u (bufs=1: u and
                # u+2 share; u+1's compute covers the WAR on u's drain)
                ps_t = [
                    pp.tile([128, 2048], F32, tag=f"ps{i}", bufs=1,
                            name=f"ps{i}")
                    for i in range(2)
                ]
                scr = pool.tile([128, 64], BF16, tag="scr")
                nc.gpsimd.memset(scr[:], 0.0)
                # warmup into unused cols 848-911 of ps0 bank 1
                for _ in range(2):
                    nc.tensor.matmul(
                        ps_t[0][0:32, 848:912], scr[0:32, :32],
                        scr[0:32, :64], start=True, stop=True,
                        tile_position=(0, 0),
                    )

                w_sb = pool.tile([128, 36 * MH], BF16, tag="w", name="w_sb")
                x_sb = pool.tile([128, XFREE], BF16, tag="x", name="x_sb")
                o_sb = pool.tile([128, U * NQ * NW], BF16, tag="o",
                                 name="o_sb")
                nc.sync.dma_start(
                    out=x_sb[:, 0 : 2 * XROW], in_=xw_d[:, 0 : 2 * XROW]
                )
                nc.gpsimd.dma_start(
                    out=w_sb[:, 0 : 12 * MH], in_=wt_d[:, 0 : 12 * MH]
                )
                nc.sync.dma_start(
                    out=x_sb[:, 2 * XROW : 5 * XROW],
                    in_=xw_d[:, 2 * XROW : 5 * XROW],
                )
                nc.gpsimd.dma_start(
                    out=w_sb[:, 12 * MH :], in_=wt_d[:, 12 * MH :]
                )
                nc.sync.dma_start(
                    out=x_sb[:, 5 * XROW :], in_=xw_d[:, 5 * XROW :]
                )

                def lhsT_for(pos9, q, h):
                    rows = slice(64 * h, 64 * h + KH)
                    blk = (pos9 * NQ + q) * MH
                    return w_sb[rows, blk : blk + MH]

                def rhs_for(u, i0, i1, q, h):
                    return (
                        x_sb[
                            64 * h : 64 * h + KH,
                            (u + i0 - 1) * XROW : (u + i0) * XROW,
                        ]
                        .rearrange("p (v q w) -> p v q w", v=V + 2, q=NQ)
                        [:, i1 : i1 + V, q, :]
                    )

                def pcol(h, q):
                    return 1024 * h + 512 * (q // 2) + 168 * (q % 2)

                for u in range(U):
                    pst = ps_t[u % 2]
                    shifts = [
                        (pos9, i0, i1)
                        for pos9, (i0, i1) in enumerate(W_ORDER)
                        if 1 <= u + i0 <= 7
                    ]
                    n = len(shifts)
                    for idx, (pos9, i0, i1) in enumerate(shifts):
                        for q in range(NQ):
                            for h in range(2):
                                nc.tensor.matmul(
                                    pst[
                                        64 * h : 64 * h + MH,
                                        pcol(h, q) : pcol(h, q) + NW,
                                    ],
                                    lhsT_for(pos9, q, h),
                                    rhs_for(u, i0, i1, q, h),
                                    # start clears the whole PSUM bank:
                                    # only the bank's first writer (q0/q2)
                                    # sets it; q1/q3 init via has_written
                                    start=(idx == 0 and q % 2 == 0),
                                    stop=(idx == n - 1),
                                    skip_group_check=True,
                                )
                    # evacuate: one strided op per half (4 q-planes)
                    ucols = slice(u * NQ * NW, (u + 1) * NQ * NW)
                    nc.scalar.activation(
                        out=o_sb[0:MH, ucols].rearrange(
                            "p (a b w) -> p a b w", a=2, b=2
                        ),
                        in_=pst[0:MH, 0:1024]
                        .rearrange("p (a w) -> p a w", a=2)[:, :, 0:336]
                        .rearrange("p a (b w) -> p a b w", b=2),
                        func=mybir.ActivationFunctionType.Identity,
                    )
                    nc.vector.tensor_scalar_add(
                        out=o_sb[64 : 64 + MH, ucols].rearrange(
                            "p (a b w) -> p a b w", a=2, b=2
                        ),
                        in0=pst[64 : 64 + MH, 1024:2048]
                        .rearrange("p (a w) -> p a w", a=2)[:, :, 0:336]
                        .rearrange("p a (b w) -> p a b w", b=2),
                        scalar1=0.0,
                    )
                    ring = nc.sync if u % 2 == 0 else nc.gpsimd
                    ring.dma_start(
                        out=out_d[:, ucols], in_=o_sb[:, ucols]
                    )

    nc.compile()
    return nc


def _get_nc():
    if "nc" not in _built:
        _built["nc"] = _build_nc()
    return _built["nc"]


def _build_weight_inputs(conv):
    Wr = conv.reshape(OC, 3, 3, 3, 3, C).astype(np.float32)
    # G-transform over i3: g_hat[q] per (o, i0, i1, i2, c)
    g0, g1, g2 = Wr[..., 0, :], Wr[..., 1, :], Wr[..., 2, :]
    gh = [g0, (g0 + g1 + g2) / 2, (g0 - g1 + g2) / 2, g2]
    # wt[64h + s_rel*8 + c, (pos9*4 + q)*48 + t_rel*8 + o]
    wt = np.zeros((128, 36, MH), np.float32)
    for half in range(2):
        for t_rel in range(THH):
            for d in range(3):
                s_rel = t_rel + d
                if s_rel >= SH:
                    continue
                for pos9, (i0, i1) in enumerate(W_ORDER):
                    for q in range(NQ):
                        p0 = 64 * half + s_rel * 8
                        wt[p0 : p0 + 8, pos9 * NQ + q,
                           t_rel * 8 : t_rel * 8 + 8] = gh[q][
                            :, i0, i1, d, :
                        ].T
    return np.ascontiguousarray(
        wt.reshape(128, 36 * MH).astype(ml_dtypes.bfloat16)
    )


def _build_x_inputs(x):
    xh = np.zeros((B, C, U, V, H + 2, W), np.float32)
    xh[:, :, :, :, 1 : H + 1, :] = x
    xs = []
    for core in range(NCORES):
        b, hc = divmod(core, NCHUNKS)
        slab = xh[b, :, :, :, hc * TH : hc * TH + S, :]  # (C,U,V,14,W)
        xc = np.zeros((C, S, U, V + 2, W + 2), np.float32)
        xc[:, :, :, 1 : V + 1, 1 : W + 1] = slab.transpose(0, 3, 1, 2, 4)
        # forward Winograd transform along w (padded idx: w = idx - 1)
        # tile k inputs d = xpad[2k .. 2k+3]
        d0 = xc[..., 0:48:2]
        d1 = xc[..., 1:49:2]
        d2 = xc[..., 2:50:2]
        d3 = xc[..., 3:50:2]
        xq = np.stack(
            [d0 - d2, d1 + d2, d2 - d1, d1 - d3], axis=-2
        )  # (C, S, U, V+2, 4, 24)
        sm = xq.transpose(1, 0, 2, 3, 4, 5)  # (S, C, U, V+2, 4, 24)
        x128 = np.empty((128, XFREE), np.float32)
        x128[0:64] = sm[0:SH].reshape(KH, XFREE)
        x128[64:128] = sm[THH : THH + SH].reshape(KH, XFREE)
        xs.append(
            np.ascontiguousarray(x128.astype(ml_dtypes.bfloat16))
        )
    return xs


def kernel(x, conv, bias):
    from concourse.bass_utils import run_bass_kernel_spmd

    nc = _get_nc()
    wt = _build_weight_inputs(np.asarray(conv))
    xs = _build_x_inputs(np.asarray(x, dtype=np.float32))
    in_maps = [{"xw": xc, "wt": wt} for xc in xs]
    res = run_bass_kernel_spmd(nc, in_maps, core_ids=list(range(NCORES)))

    bias = np.asarray(bias, dtype=np.float32)
    out = np.empty((B, OC, U, V, H, W), np.float32)
    for core in range(NCORES):
        b, hc = divmod(core, NCHUNKS)
        raw = np.asarray(res.results[core]["out"], dtype=np.float32)
        # rows: half*64 + t_rel*8 + o ; cols: (u, q, v, k)
        raw = raw.reshape(128, U, NQ, V, NT)
        y = np.concatenate(
            [raw[0:MH], raw[64 : 64 + MH]], axis=0
        ).reshape(TH, OC, U, NQ, V, NT)
        ev = y[:, :, :, 0] + y[:, :, :, 1] + y[:, :, :, 2]
        od = y[:, :, :, 1] - y[:, :, :, 2] - y[:, :, :, 3]
        w2 = np.stack([ev, od], axis=-1).reshape(TH, OC, U, V, W)
        out[b, :, :, :, hc * TH : (hc + 1) * TH, :] = (
            w2.transpose(1, 2, 3, 0, 4) + bias[:, None, None, None, None]
        )
    return out


# revision 16
# speedup vs baseline: 1.8444x; 1.5779x over previous
"""Trainium2 Bass kernel for 4D convolution (3x3x3x3, pad 1, stride 1).

Problem: x (2, 8, 7, 7, 48, 48) f32, conv (8, 648) f32, bias (8,) f32
         -> out (2, 8, 7, 7, 48, 48) f32.

Sharding: 8 cores = (batch b in {0,1}) x (h-chunk hc in {0..3}, 12 rows).

Per core: four-way concurrent banded matmuls (bf16), pair-u drains,
per-pair streamed output, DMA ring choreography (x split across both
rings; mid-kernel outs on SP; final pair split across rings).

Each core's h-chunk (12 rows) splits into two 6-row sub-chunks:
  LO: outputs t 0..5,  contraction window s 0..7   -> partitions 0..63,  PE rows 0-63
  HI: outputs t 6..11, contraction window s 6..13  -> partitions 64..127, PE rows 64-127
Per (u, shift) the two K=64, M=48 matmuls occupy disjoint row-group pairs of
the PE array and stream concurrently -> ~2x matmul wall-clock vs one
K=112/M=96 matmul. Outputs accumulate in separate PSUM banks (cols 0-47),
accumulated into pair-u PSUM tiles ([128,1024], two banks: u-even at
cols 0-335, u-odd at 512-847), drained two u-rows at a time by one
strided bias-activation (ACT) plus one strided partial-sum add (DVE),
and streamed to DRAM per pair. Outputs leave as bf16 (host upcasts).

Partition layout is s-major (p = s*8 + c); rows s6, s7 are duplicated across
the two halves (128 rows vs 112 unique).
"""

import sys

if "/opt/trn_rl_repo" not in sys.path:
    sys.path.insert(0, "/opt/trn_rl_repo")

import numpy as np
import ml_dtypes

B, C, OC = 2, 8, 8
U, V, H, W = 7, 7, 48, 48
TH = 12
THH = 6             # rows per half
S = TH + 2
SH = THH + 2        # window rows per half
KH = C * SH         # 64  contraction per half
MH = OC * THH       # 48  outputs per half
NCHUNKS = H // TH
NCORES = B * NCHUNKS
NCOL = V * W        # 336
XROW = (V + 2) * (W + 2)  # 450
XFREE = U * XROW

SH_ORDER = [
    (i0, i1, i3) for i0 in (1, 2, 0) for i1 in range(3) for i3 in range(3)
]

N_WARMUP_MM = 4

_built = {}


def _build_nc(reps=None):
    import contextlib

    import concourse.bacc as bacc
    import concourse.mybir as mybir
    from concourse.tile import TileContext

    BF16 = mybir.dt.bfloat16
    F32 = mybir.dt.float32

    nc = bacc.Bacc(
        "TRN2", target_bir_lowering=False, debug=False, num_devices=NCORES
    )
    xw_d = nc.dram_tensor("xw", [128, XFREE], BF16, kind="ExternalInput")
    wt_d = nc.dram_tensor("wt", [128, 27 * MH], BF16, kind="ExternalInput")
    bias_d = nc.dram_tensor("bias", [128, 1], F32, kind="ExternalInput")
    out_d = nc.dram_tensor("out", [2 * MH, U * NCOL], BF16, kind="ExternalOutput")

    with TileContext(nc) as tc:
        with (
            tc.tile_pool(name="sbuf", bufs=1) as pool,
            tc.tile_pool(name="psum", bufs=1, space="PSUM") as pp,
        ):
            loop = tc.For_i(0, reps, 1) if reps is not None else contextlib.nullcontext()
            with loop:
                scr = pool.tile([128, 512], BF16, tag="scr")
                nc.gpsimd.memset(scr[:], 0.0)
                ps_w = pp.tile([128, 1024], F32, tag="ps0", bufs=2)
                for _ in range(N_WARMUP_MM):
                    nc.tensor.matmul(
                        ps_w[:, :64], scr[:, :128], scr[:, :64], start=True,
                        stop=True,
                    )

                w_first = pool.tile([128, MH], BF16, tag="wf", name="w_first")
                w_sb = pool.tile([128, 26 * MH], BF16, tag="w", name="w_sb")
                x_sb = pool.tile([128, XFREE], BF16, tag="x", name="x_sb")
                b_sb = pool.tile([128, 1], F32, tag="b")
                nc.scalar.dma_start(out=w_first[:], in_=wt_d[:, 0:MH])
                nc.sync.dma_start(
                    out=x_sb[:, 0 : 3 * XROW], in_=xw_d[:, 0 : 3 * XROW]
                )
                nc.scalar.dma_start(out=w_sb[:], in_=wt_d[:, MH:])
                nc.scalar.dma_start(
                    out=x_sb[:, 3 * XROW :], in_=xw_d[:, 3 * XROW :]
                )
                nc.scalar.dma_start(out=b_sb[:], in_=bias_d[:])

                def lhsT_for(pos, half):
                    rows = slice(64 * half, 64 * half + KH)
                    if pos == 0:
                        return w_first[rows, :]
                    return w_sb[rows, (pos - 1) * MH : pos * MH]

                def rhs_for(u, i0, i1, i3, half):
                    return (
                        x_sb[
                            64 * half : 64 * half + KH,
                            (u + i0 - 1) * XROW : (u + i0) * XROW,
                        ]
                        .rearrange("p (v w) -> p v w", v=V + 2)
                        [:, i1 : i1 + V, i3 : i3 + W]
                    )

                npairs = (U + 1) // 2
                pair_ps = [
                    [
                        pp.tile(
                            [128, 1024],
                            F32,
                            tag=f"ps{h}",
                            bufs=2,
                            name=f"ps{h}_pr{pr}",
                        )
                        for h in range(2)
                    ]
                    for pr in range(npairs)
                ]

                def ps_slice(u, h, p):
                    off = 512 * (u % 2)
                    return pair_ps[u // 2][h][
                        64 * p : 64 * p + MH, off : off + NCOL
                    ]
                # halves live at partition bases 0 and 64 (engine operands
                # must sit at 32-aligned bases; 48 is rejected by walrus)
                o_sb = pool.tile([128, U * NCOL], BF16, tag="o", name="o_sb")

                for u in range(U):
                    shifts = [
                        (pos, i0, i1, i3)
                        for pos, (i0, i1, i3) in enumerate(SH_ORDER)
                        if 1 <= u + i0 <= 7
                    ]
                    npar = [
                        len([i for i in range(len(shifts)) if i % 2 == p])
                        for p in range(2)
                    ]
                    cnt = [0, 0]
                    for idx, (pos, i0, i1, i3) in enumerate(shifts):
                        p = idx % 2
                        for h in range(2):
                            nc.tensor.matmul(
                                ps_slice(u, h, p),
                                lhsT_for(pos, h),
                                rhs_for(u, i0, i1, i3, h),
                                start=(cnt[p] == 0),
                                stop=(cnt[p] == npar[p] - 1),
                            )
                        cnt[p] += 1
                    if u % 2 == 1 or u == U - 1:
                        # drain the completed pair: one strided ACT pass
                        # (even parity + bias) and one DVE pass (+= odd),
                        # covering both u-rows of the pair at once
                        pr = u // 2
                        nu = 1 if u == U - 1 and U % 2 == 1 else 2
                        ucols = slice((u - nu + 1) * NCOL, (u + 1) * NCOL)
                        for h in range(2):
                            pst = pair_ps[pr][h]
                            pin_even = (
                                pst[0:MH, : 512 * nu]
                                .rearrange("p (n x) -> p n x", n=nu)
                                [:, :, 0:NCOL]
                            )
                            pin_odd = (
                                pst[64 : 64 + MH, : 512 * nu]
                                .rearrange("p (n x) -> p n x", n=nu)
                                [:, :, 0:NCOL]
                            )
                            osl = (
                                o_sb[64 * h : 64 * h + MH, ucols]
                                .rearrange("p (n x) -> p n x", n=nu)
                            )
                            nc.scalar.activation(
                                out=osl,
                                in_=pin_even,
                                func=mybir.ActivationFunctionType.Identity,
                                bias=b_sb[0:MH, :],
                            )
                            nc.vector.scalar_tensor_tensor(
                                out=osl,
                                in0=pin_odd,
                                scalar=0.0,
                                in1=osl,
                                op0=mybir.AluOpType.add,
                                op1=mybir.AluOpType.add,
                            )
                        # stream this pair's output out as soon as it
                        # drains; alternate rings to spread HWDGE work
                        last = u == U - 1
                        for h in range(2):
                            # mid-kernel outs ride the SP ring (idle after
                            # x chunk 1; keeps descriptor gen off the ACT
                            # sequencer during drains); the final pair
                            # splits across rings so its two completion
                            # waits run in parallel
                            ring = nc.scalar if (last and h == 1) else nc.sync
                            ring.dma_start(
                                out=out_d[h * MH : (h + 1) * MH, ucols],
                                in_=o_sb[64 * h : 64 * h + MH, ucols],
                            )

    nc.compile()
    return nc


def _get_nc():
    if "nc" not in _built:
        _built["nc"] = _build_nc()
    return _built["nc"]


def _build_weight_inputs(conv, bias):
    Wr = conv.reshape(OC, 3, 3, 3, 3, C).astype(np.float32)
    # wt[p, pos, t_rel*8 + o]; p = 64*half + (s_rel*8 + c); s = s_rel + 6*half
    wt = np.zeros((128, 27, MH), np.float32)
    for half in range(2):
        for t_rel in range(THH):
            for d in range(3):
                s_rel = t_rel + d
                if s_rel >= SH:
                    continue
                for pos, (i0, i1, i3) in enumerate(SH_ORDER):
                    # rows p = 64*half + s_rel*8 + c ; cols t_rel*8 + o
                    p0 = 64 * half + s_rel * 8
                    wt[p0 : p0 + 8, pos, t_rel * 8 : t_rel * 8 + 8] = Wr[
                        :, i0, i1, d, i3, :
                    ].T
    wt = np.ascontiguousarray(
        wt.reshape(128, 27 * MH).astype(ml_dtypes.bfloat16)
    )
    # bias rows: halves at partition bases 0 and 64, (t_rel, o) within
    bias_in = np.zeros((128, 1), np.float32)
    half_bias = np.tile(bias.astype(np.float32), THH).reshape(MH, 1)
    bias_in[0:MH] = half_bias
    bias_in[64 : 64 + MH] = half_bias
    return wt, bias_in


def _build_x_inputs(x):
    xh = np.zeros((B, C, U, V, H + 2, W), np.float32)
    xh[:, :, :, :, 1 : H + 1, :] = x
    xs = []
    for core in range(NCORES):
        b, hc = divmod(core, NCHUNKS)
        slab = xh[b, :, :, :, hc * TH : hc * TH + S, :]  # (C, U, V, S, W)
        xc = np.zeros((C, S, U, V + 2, W + 2), np.float32)
        xc[:, :, :, 1 : V + 1, 1 : W + 1] = slab.transpose(0, 3, 1, 2, 4)
        sm = xc.transpose(1, 0, 2, 3, 4)  # (S, C, U, V+2, W+2)
        x128 = np.empty((128, XFREE), np.float32)
        x128[0:64] = sm[0:SH].reshape(KH, XFREE)
        x128[64:128] = sm[THH : THH + SH].reshape(KH, XFREE)
        xs.append(
            np.ascontiguousarray(x128.astype(ml_dtypes.bfloat16))
        )
    return xs


def kernel(x, conv, bias):
    from concourse.bass_utils import run_bass_kernel_spmd

    nc = _get_nc()
    wt, bias_in = _build_weight_inputs(np.asarray(conv), np.asarray(bias))
    xs = _build_x_inputs(np.asarray(x, dtype=np.float32))
    in_maps = [{"xw": xc, "wt": wt, "bias": bias_in} for xc in xs]
    res = run_bass_kernel_spmd(nc, in_maps, core_ids=list(range(NCORES)))

    out = np.empty((B, OC, U, V, H, W), np.float32)
    for core in range(NCORES):
        b, hc = divmod(core, NCHUNKS)
        # out rows: (half, t_rel, o) -> t = half*6 + t_rel
        r = np.asarray(
            res.results[core]["out"], dtype=np.float32
        ).reshape(TH, OC, U, V, W)
        out[b, :, :, :, hc * TH : (hc + 1) * TH, :] = r.transpose(1, 2, 3, 0, 4)
    return out

